# revision 9
# baseline (speedup 1.0000x reference)
"""Trainium2 Bass kernel for DescartesExtension (order-2, with replacement).

out[b, k] = x[b, ii[k]] * x[b, jj[k]] with (ii, jj) = triu_indices(D), i.e.
the output row is the concatenation over i of x[b, i] * x[b, i:D].

Sharding: data-parallel over the batch dim — 1024 rows / 8 cores = 128 rows
per core, which is exactly one SBUF partition tile. Per core the kernel:
  1. loads its [128, 512] x shard into SBUF (one tiny DMA),
  2. for each i computes the segment x[:, i] * x[:, i:] with a per-partition
     broadcast multiply (VectorE tensor_scalar or ScalarE activation-Copy
     with a [128,1] scale operand), packing segments contiguously into
     ~8K-element SBUF chunks,
  3. DMAs each chunk (~4.3 MB) to its slice of the output row via HWDGE.

The problem is HBM-write bound (538 MB total output, 2 MB input), so the
multiply work is split across VectorE and ScalarE only so that compute stays
safely under the DMA time and fully overlaps with it.
"""

import numpy as np

N_CORES = 8
B = 1024
D = 512
K = D * (D + 1) // 2  # 131328
BS = B // N_CORES  # 128 rows per core = one partition tile

# Output is produced in segment-aligned chunks. Small ramp chunks first so
# the first DMA issues within ~1 segment of compute; big steady chunks after
# so per-partition DMA descriptors are large (64+ KB) and near line-rate.
RAMP_TARGETS = [512, 1024, 2048, 4096]
STEADY_TARGET = 16384
RAMP_BUFS = 2
STEADY_BUFS = 2

_CACHE = {}


def _segments():
    lengths = [D - i for i in range(D)]
    offs = [0]
    for ln in lengths:
        offs.append(offs[-1] + ln)
    return lengths, offs


def _chunks(lengths):
    """Segment-aligned chunks: ramp targets first, then steady-state."""
    targets = list(RAMP_TARGETS)
    chunks = []
    i = 0
    off = 0
    while i < D:
        target = targets.pop(0) if targets else STEADY_TARGET
        s = i
        clen = 0
        while i < D and clen < target:
            clen += lengths[i]
            i += 1
        chunks.append((s, i, off, clen))
        off += clen
    return chunks


def _engine_split(lengths, chunks):
    """Greedy static balance between VectorE and ScalarE at chunk granularity.

    A whole chunk goes to one engine so the chunk's output DMA waits on a
    single semaphore (HWDGE DMA instructions only support one sync-wait).

    DVE fp32 tensor_scalar: ~(58 + L/2) cycles @0.96 GHz when the 2x two-port
    mode kicks in (even length), (58 + L) otherwise. ACT activation-Copy:
    ~(224 + L) cycles @1.2 GHz.
    """
    t_v = 0.0
    t_s = 0.0
    assign = []
    for s, e, _off0, _clen in chunks:
        c_v = sum(
            (58 + (ln // 2 if ln % 2 == 0 else ln)) / 0.96
            for ln in lengths[s:e]
        )
        c_s = sum((224 + ln) / 1.2 for ln in lengths[s:e])
        if t_v + c_v <= t_s + c_s:
            assign.append("v")
            t_v += c_v
        else:
            assign.append("s")
            t_s += c_s
    return assign


def _build():
    if "nc" in _CACHE:
        return _CACHE["nc"]
    import concourse.tile as tile
    from concourse import bacc, mybir

    nc = bacc.Bacc("TRN2", debug=False)
    x_ap = nc.dram_tensor("x", [BS, D], mybir.dt.float32, kind="ExternalInput").ap()
    out_ap = nc.dram_tensor(
        "out", [BS, K], mybir.dt.float32, kind="ExternalOutput"
    ).ap()

    lengths, offs = _segments()
    chunks = _chunks(lengths)
    assign = _engine_split(lengths, chunks)
    n_ramp = len(RAMP_TARGETS)
    ramp_max = max(c[3] for c in chunks[:n_ramp])
    steady_max = max(c[3] for c in chunks[n_ramp:])

    with tile.TileContext(nc) as tc:
        with (
            tc.tile_pool(name="xp", bufs=1) as xp,
            tc.tile_pool(name="rp", bufs=RAMP_BUFS) as rp,
            tc.tile_pool(name="op", bufs=STEADY_BUFS) as op,
        ):
            xt = xp.tile([BS, D], mybir.dt.float32)
            nc.sync.dma_start(xt[:], x_ap[:])
            for ci, (s, e, off0, clen) in enumerate(chunks):
                if ci < n_ramp:
                    ot = rp.tile([BS, ramp_max], mybir.dt.float32, tag="ramp")
                else:
                    ot = op.tile([BS, steady_max], mybir.dt.float32, tag="out")
                for i in range(s, e):
                    ln = lengths[i]
                    dst = ot[:, offs[i] - off0 : offs[i] - off0 + ln]
                    src = xt[:, i:D]
                    scal = xt[:, i : i + 1]
                    if assign[ci] == "v":
                        nc.vector.tensor_scalar_mul(dst, src, scal)
                    else:
                        nc.scalar.activation(
                            dst, src, mybir.ActivationFunctionType.Copy, scale=scal
                        )
                # DMA issued from the engine that computed the chunk: same-
                # engine FIFO ordering (no cross-engine wait) and the two
                # HWDGE rings (SP for DVE chunks, ACT for ACT chunks) split
                # the queue load.
                dma_eng = nc.sync if assign[ci] == "v" else nc.scalar
                dma_eng.dma_start(out_ap[:, off0 : off0 + clen], ot[:, :clen])

    nc.compile()
    _CACHE["nc"] = nc
    return nc


def _run(x, trace=False):
    from concourse.bass_utils import run_bass_kernel_spmd

    nc = _build()
    x = np.ascontiguousarray(x, dtype=np.float32)
    assert x.shape == (B, D), x.shape
    in_maps = [{"x": x[c * BS : (c + 1) * BS]} for c in range(N_CORES)]
    res = run_bass_kernel_spmd(nc, in_maps, list(range(N_CORES)), trace=trace)
    out = np.concatenate([res.results[c]["out"] for c in range(N_CORES)], axis=0)
    return out, res


def kernel(x):
    return _run(x)[0]
